# revision 9
# baseline (speedup 1.0000x reference)
"""Distributed Trainium2 kernel for GNN message passing (COO SpMM + dense head).

out = relu((A @ x) @ W[:128] + x @ W[128:])   with A given as COO (rows, cols, vals)

Strategy (8 NeuronCores, SPMD single graph):
  - Rows (destinations) sharded across cores: core c owns rows [c*12500, (c+1)*12500).
  - x is replicated to every core's DRAM via its input map (no collectives needed).
  - Host-side index preprocessing only (sorting / padding / layout): per core the
    edge list is sorted by col-chunk (4 chunks of 25000 so gather indices fit in
    int16), padded to shared per-chunk capacities so all 8 cores run the same graph.
  - On device per tile of edges: hardware gather x[col] (SWDGE dma_gather),
    scale by vals (VectorE broadcast multiply), hardware scatter-add into a DRAM
    h accumulator (SWDGE dma_scatter_add).
  - Dense head: per 128-row group, h.T via PE transpose, out = relu(hT.T@W1 + xT.T@W2)
    accumulated in PSUM, relu on ScalarE, DMA out.
"""

import sys

if "/opt/trn_rl_repo" not in sys.path:
    sys.path.insert(0, "/opt/trn_rl_repo")

import numpy as np

N_NODES = 100000
N_EDGES = 600000
D = 128
OUT = 128
P = 128
NCORES = 8
RPC = N_NODES // NCORES          # 12500 rows per core
NCHUNK = 4
CHUNK = N_NODES // NCHUNK        # 25000 (< 32768 so int16 gather idx works)
TILE_E = 1024                    # max edges per SWDGE call (1024-descriptor ring limit)
H_PAD = 12800                    # padded row count per parity stripe
MAXROUNDS = 16                   # upper bound on per-(chunk,row) multiplicity / 2

_compiled = {}


def _prep(adj_rows, adj_cols, adj_vals):
    """Per-core uniform-shape gather/scatter metadata (int/layout work only).

    The HW scatter-add races on duplicate destinations within one call, so we
    stripe h over 2 parities (dest = row*2 + parity) and deal each row's j-th
    edge within a (core, chunk) group to round j//2, parity j%2.  Every
    (chunk, round) group then has unique (row, parity) destinations, and
    groups run as separate serialized scatter calls.
    """
    rows = np.asarray(adj_rows).astype(np.int64)
    cols = np.asarray(adj_cols).astype(np.int64)
    vals = np.asarray(adj_vals).astype(np.float32)

    per_core = []
    sizes_all = np.zeros((NCORES, NCHUNK, MAXROUNDS), np.int64)
    for c in range(NCORES):
        m = (rows >= c * RPC) & (rows < (c + 1) * RPC)
        r = rows[m] - c * RPC
        co = cols[m]
        v = vals[m]
        ch = co // CHUNK
        # occurrence index of each edge within its (chunk, row) set
        o = np.lexsort((r, ch))
        r, co, v, ch = r[o], co[o], v[o], ch[o]
        key = ch * RPC + r
        # run-length occurrence index over the sorted keys
        change = np.empty(len(key), bool)
        if len(key):
            change[0] = True
            change[1:] = key[1:] != key[:-1]
        run_starts = np.flatnonzero(change)
        occ = np.arange(len(key)) - np.repeat(run_starts, np.diff(
            np.append(run_starts, len(key))))
        rnd = occ // 2
        assert rnd.max(initial=0) < MAXROUNDS
        parity = (occ % 2).astype(np.int64)
        # order by (chunk, round, row)
        o2 = np.lexsort((r, rnd, ch))
        r, co, v, ch, rnd, parity = (a[o2] for a in (r, co, v, ch, rnd, parity))
        for k in range(NCHUNK):
            mk = ch == k
            sizes_all[c, k] = np.bincount(rnd[mk], minlength=MAXROUNDS)
        per_core.append((r, co % CHUNK, v, ch, rnd, parity))

    caps = ((sizes_all.max(axis=0) + 127) // 128) * 128     # [NCHUNK, MAXROUNDS]
    T = int(caps.sum())

    # static call list: (chunk, dram_offset, n) per call
    calls = []
    off = 0
    for k in range(NCHUNK):
        for j in range(MAXROUNDS):
            cap = int(caps[k, j])
            for t0 in range(0, cap, TILE_E):
                calls.append((k, off + t0, min(TILE_E, cap - t0)))
            off += cap

    gidx_w = np.zeros((NCORES, P, T // 16), np.int16)
    sidx_w = np.zeros((NCORES, P, T // 16), np.int16)
    sval_w = np.zeros((NCORES, P, T // 128), np.float32)
    offs = np.concatenate([[0], np.cumsum(caps.reshape(-1))]).astype(np.int64)

    for c in range(NCORES):
        r, coi, v, ch, rnd, parity = per_core[c]
        gi = np.zeros(T, np.int16)
        si = np.zeros(T, np.int16)
        sv = np.zeros(T, np.float32)
        s = 0
        for k in range(NCHUNK):
            for j in range(MAXROUNDS):
                n = int(sizes_all[c, k, j])
                d0 = int(offs[k * MAXROUNDS + j])
                gi[d0:d0 + n] = coi[s:s + n]
                si[d0:d0 + n] = r[s:s + n] * 2 + parity[s:s + n]
                sv[d0:d0 + n] = v[s:s + n]
                s += n
        # wrap for the SWDGE index layout: idx i -> [i%16, i//16], replicated x8
        gidx_w[c] = np.tile(gi.reshape(-1, 16).T, (8, 1))
        sidx_w[c] = np.tile(si.reshape(-1, 16).T, (8, 1))
        # vals layout: edge i -> [i%128, i//128]
        sval_w[c] = sv.reshape(-1, 128).T

    return tuple(calls), gidx_w, sidx_w, sval_w


def _build(calls):
    from concourse import bass, mybir, tile, bacc
    from concourse.masks import make_identity

    f32 = mybir.dt.float32
    i16 = mybir.dt.int16
    T = int(sum(n for _, _, n in calls))
    T = max(e0 + n for _, e0, n in calls)

    nc = bacc.Bacc("TRN2", target_bir_lowering=False, debug=False)

    x_d = nc.dram_tensor("x", [N_NODES, D], f32, kind="ExternalInput")
    xT_d = nc.dram_tensor("xlocT", [D, RPC], f32, kind="ExternalInput")
    w_d = nc.dram_tensor("W", [2 * D, OUT], f32, kind="ExternalInput")
    gidx_d = nc.dram_tensor("gidx", [P, T // 16], i16, kind="ExternalInput")
    sidx_d = nc.dram_tensor("sidx", [P, T // 16], i16, kind="ExternalInput")
    sval_d = nc.dram_tensor("svals", [P, T // 128], f32, kind="ExternalInput")
    out_d = nc.dram_tensor("out", [RPC, OUT], f32, kind="ExternalOutput")
    h_d = nc.dram_tensor("h_acc", [2 * H_PAD, D], f32)   # parity-striped

    relu = mybir.ActivationFunctionType.Relu

    with tile.TileContext(nc) as tc:
        with tc.tile_pool(name="const", bufs=1) as constp, \
             tc.tile_pool(name="mess", bufs=3) as messp, \
             tc.tile_pool(name="meta", bufs=3) as metap, \
             tc.tile_pool(name="dense", bufs=4) as densep, \
             tc.tile_pool(name="psum", bufs=2, space="PSUM") as psump:

            ident = constp.tile([P, P], f32)
            make_identity(nc, ident[:])
            w1 = constp.tile([D, OUT], f32)
            nc.sync.dma_start(out=w1[:], in_=w_d[:D, :])
            w2 = constp.tile([D, OUT], f32)
            nc.sync.dma_start(out=w2[:], in_=w_d[D:, :])

            # zero the striped h accumulator (2*H_PAD = 25600 rows)
            zblk = constp.tile([P, 8, D], f32)
            nc.vector.memset(zblk[:], 0.0)
            for b in range(2 * H_PAD // 1024):
                dst = h_d[b * 1024:(b + 1) * 1024, :].rearrange(
                    "(a p) d -> p a d", p=P)
                nc.scalar.dma_start(out=dst, in_=zblk[:])

            # ---- SpMM phase: gather -> scale -> scatter-add ----
            for k, e0, n in calls:
                ns = n // 128
                x_chunk = x_d[k * CHUNK:(k + 1) * CHUNK, :]
                gi = metap.tile([P, TILE_E // 16], i16, tag="gi")
                nc.sync.dma_start(
                    out=gi[:, :n // 16],
                    in_=gidx_d[:, e0 // 16:(e0 + n) // 16])
                mv = messp.tile([P, TILE_E // 128, D], f32, tag="mess")
                nc.gpsimd.dma_gather(
                    mv[:, :ns, :], x_chunk, gi[:, :n // 16], n, n, D)
                sv = metap.tile([P, TILE_E // 128], f32, tag="sv")
                nc.sync.dma_start(
                    out=sv[:, :ns],
                    in_=sval_d[:, e0 // 128:(e0 + n) // 128])
                nc.vector.tensor_tensor(
                    out=mv[:, :ns, :], in0=mv[:, :ns, :],
                    in1=sv[:, :ns, None].to_broadcast([P, ns, D]),
                    op=mybir.AluOpType.mult)
                si = metap.tile([P, TILE_E // 16], i16, tag="si")
                nc.scalar.dma_start(
                    out=si[:, :n // 16],
                    in_=sidx_d[:, e0 // 16:(e0 + n) // 16])
                nc.gpsimd.dma_scatter_add(
                    h_d[:], mv[:, :ns, :], si[:, :n // 16], n, n, D)

            # ---- dense head: out = relu(h @ W1 + x @ W2) ----
            h_pairs = h_d[:].rearrange("(a two) d -> a two d", two=2)
            ngroups = (RPC + P - 1) // P       # 98 (97 full + one 84-row tail)
            for g in range(ngroups):
                g0 = g * P
                rsz = min(P, RPC - g0)
                hb = densep.tile([P, D], f32, tag="hb")
                nc.sync.dma_start(out=hb[:rsz, :],
                                  in_=h_pairs[g0:g0 + rsz, 0, :])
                hc = densep.tile([P, D], f32, tag="hc")
                nc.sync.dma_start(out=hc[:rsz, :],
                                  in_=h_pairs[g0:g0 + rsz, 1, :])
                nc.vector.tensor_add(out=hb[:rsz, :], in0=hb[:rsz, :],
                                     in1=hc[:rsz, :])
                pt = psump.tile([P, P], f32, tag="pt")
                nc.tensor.transpose(pt[:, :rsz], hb[:rsz, :], ident[:rsz, :rsz])
                hT = densep.tile([P, P], f32, tag="hT")
                nc.vector.tensor_copy(hT[:, :rsz], pt[:, :rsz])
                xT = densep.tile([P, P], f32, tag="xT")
                nc.sync.dma_start(out=xT[:, :rsz], in_=xT_d[:, g0:g0 + rsz])
                po = psump.tile([P, OUT], f32, tag="po")
                nc.tensor.matmul(po[:rsz, :], hT[:, :rsz], w1[:],
                                 start=True, stop=False)
                nc.tensor.matmul(po[:rsz, :], xT[:, :rsz], w2[:],
                                 start=False, stop=True)
                ob = densep.tile([P, OUT], f32, tag="ob")
                nc.scalar.activation(ob[:rsz, :], po[:rsz, :], relu)
                nc.scalar.dma_start(out=out_d[g0:g0 + rsz, :], in_=ob[:rsz, :])

    nc.compile()
    return nc


def _get_nc(calls):
    nc = _compiled.get(calls)
    if nc is None:
        nc = _build(calls)
        _compiled[calls] = nc
    return nc


def _make_in_maps(x, W, calls, gidx_w, sidx_w, sval_w):
    x = np.ascontiguousarray(np.asarray(x, np.float32))
    W = np.ascontiguousarray(np.asarray(W, np.float32))
    in_maps = []
    for c in range(NCORES):
        xloc = x[c * RPC:(c + 1) * RPC]
        in_maps.append({
            "x": x,
            "xlocT": np.ascontiguousarray(xloc.T),
            "W": W,
            "gidx": gidx_w[c],
            "sidx": sidx_w[c],
            "svals": sval_w[c],
        })
    return in_maps


def _install_trace_shims():
    """Make trace=True work in this container: provide antenv.axon_hooks
    (ctypes NTFF profiling via the axon PJRT .so) and stub the artifact
    upload (no bucket access here)."""
    import contextlib
    import ctypes
    import types

    try:
        import antenv.axon_hooks  # noqa: F401
        has_hooks = True
    except ImportError:
        has_hooks = False
    if not has_hooks:
        so_path = "/opt/axon/libaxon_pjrt.so"
        lib = ctypes.CDLL(so_path)
        if hasattr(lib, "axon_start_nrt_profile"):
            lib.axon_start_nrt_profile.argtypes = [
                ctypes.POINTER(ctypes.c_int64), ctypes.c_size_t]
            lib.axon_start_nrt_profile.restype = ctypes.c_int64
            lib.axon_stop_nrt_profile.argtypes = [ctypes.c_char_p]
            lib.axon_stop_nrt_profile.restype = ctypes.c_int64

            @contextlib.contextmanager
            def _hook(output_dir, device_ids):
                import jax
                jax.devices()
                if device_ids:
                    ids = (ctypes.c_int64 * len(device_ids))(*device_ids)
                    rc = lib.axon_start_nrt_profile(ids, len(device_ids))
                else:
                    rc = lib.axon_start_nrt_profile(None, 0)
                if rc != 0:
                    raise RuntimeError(f"axon_start_nrt_profile rc={rc}")
                try:
                    yield
                finally:
                    n = lib.axon_stop_nrt_profile(str(output_dir).encode())
                    if n <= 0:
                        print(f"ntff profile: rc={n} (no files?) at {output_dir}")

            mod = types.ModuleType("antenv.axon_hooks")
            mod.get_axon_ntff_profile_hook = lambda: _hook
            mod.set_axon_ntff_profile_hook = lambda h: None
            sys.modules["antenv.axon_hooks"] = mod

    import concourse.bass_utils as bu
    bu.upload_artifacts = lambda tmpdir: f"local:{tmpdir}"


def _run(x, adj_rows, adj_cols, adj_vals, W, trace=False):
    from concourse.bass_utils import run_bass_kernel_spmd
    if trace:
        try:
            _install_trace_shims()
        except Exception as e:  # tracing is best-effort
            print("trace shim install failed:", e)
    calls, gidx_w, sidx_w, sval_w = _prep(adj_rows, adj_cols, adj_vals)
    nc = _get_nc(calls)
    in_maps = _make_in_maps(x, W, calls, gidx_w, sidx_w, sval_w)
    res = run_bass_kernel_spmd(nc, in_maps, list(range(NCORES)), trace=trace)
    out = np.concatenate([res.results[c]["out"] for c in range(NCORES)], axis=0)
    return out, res


def kernel(x, adj_rows, adj_cols, adj_vals, W):
    out, _ = _run(x, adj_rows, adj_cols, adj_vals, W, trace=False)
    return out


# revision 10
# speedup vs baseline: 1.0022x; 1.0022x over previous
"""Distributed Trainium2 kernel for GNN message passing (COO SpMM + dense head).

out = relu((A @ x) @ W[:128] + x @ W[128:])   with A given as COO (rows, cols, vals)

Strategy (8 NeuronCores, SPMD single graph):
  - Rows (destinations) sharded across cores: core c owns rows [c*12500, (c+1)*12500).
  - x is replicated to every core's DRAM via its input map (no collectives needed).
  - Host-side index preprocessing only (sorting / padding / layout): per core the
    edge list is sorted by col-chunk (4 chunks of 25000 so gather indices fit in
    int16), padded to shared per-chunk capacities so all 8 cores run the same graph.
  - On device per tile of edges: hardware gather x[col] (SWDGE dma_gather),
    scale by vals (VectorE broadcast multiply), hardware scatter-add into a DRAM
    h accumulator (SWDGE dma_scatter_add).
  - Dense head: per 128-row group, h.T via PE transpose, out = relu(hT.T@W1 + xT.T@W2)
    accumulated in PSUM, relu on ScalarE, DMA out.
"""

import sys

if "/opt/trn_rl_repo" not in sys.path:
    sys.path.insert(0, "/opt/trn_rl_repo")

import numpy as np

N_NODES = 100000
N_EDGES = 600000
D = 128
OUT = 128
P = 128
NCORES = 8
RPC = N_NODES // NCORES          # 12500 rows per core
NCHUNK = 4
CHUNK = N_NODES // NCHUNK        # 25000 (< 32768 so int16 gather idx works)
TILE_E = 1024                    # max edges per SWDGE call (1024-descriptor ring limit)
H_PAD = 12800                    # padded row count per parity stripe
MAXROUNDS = 16                   # upper bound on per-(chunk,row) multiplicity / 2

_compiled = {}


def _prep(adj_rows, adj_cols, adj_vals):
    """Per-core uniform-shape gather/scatter metadata (int/layout work only).

    The HW scatter-add races on duplicate destinations within one call, so we
    stripe h over 2 parities (dest = row*2 + parity) and deal each row's j-th
    edge within a (core, chunk) group to round j//2, parity j%2.  Every
    (chunk, round) group then has unique (row, parity) destinations, and
    groups run as separate serialized scatter calls.
    """
    rows = np.asarray(adj_rows).astype(np.int64)
    cols = np.asarray(adj_cols).astype(np.int64)
    vals = np.asarray(adj_vals).astype(np.float32)

    per_core = []
    sizes_all = np.zeros((NCORES, NCHUNK, MAXROUNDS), np.int64)
    for c in range(NCORES):
        m = (rows >= c * RPC) & (rows < (c + 1) * RPC)
        r = rows[m] - c * RPC
        co = cols[m]
        v = vals[m]
        ch = co // CHUNK
        # occurrence index of each edge within its (chunk, row) set
        o = np.lexsort((r, ch))
        r, co, v, ch = r[o], co[o], v[o], ch[o]
        key = ch * RPC + r
        # run-length occurrence index over the sorted keys
        change = np.empty(len(key), bool)
        if len(key):
            change[0] = True
            change[1:] = key[1:] != key[:-1]
        run_starts = np.flatnonzero(change)
        occ = np.arange(len(key)) - np.repeat(run_starts, np.diff(
            np.append(run_starts, len(key))))
        rnd = occ // 2
        assert rnd.max(initial=0) < MAXROUNDS
        parity = (occ % 2).astype(np.int64)
        # order by (chunk, round, row)
        o2 = np.lexsort((r, rnd, ch))
        r, co, v, ch, rnd, parity = (a[o2] for a in (r, co, v, ch, rnd, parity))
        for k in range(NCHUNK):
            mk = ch == k
            sizes_all[c, k] = np.bincount(rnd[mk], minlength=MAXROUNDS)
        per_core.append((r, co % CHUNK, v, ch, rnd, parity))

    caps = ((sizes_all.max(axis=0) + 127) // 128) * 128     # [NCHUNK, MAXROUNDS]
    T = int(caps.sum())

    # static call list: (chunk, dram_offset, n) per call
    calls = []
    off = 0
    for k in range(NCHUNK):
        for j in range(MAXROUNDS):
            cap = int(caps[k, j])
            for t0 in range(0, cap, TILE_E):
                calls.append((k, off + t0, min(TILE_E, cap - t0)))
            off += cap

    gidx_w = np.zeros((NCORES, P, T // 16), np.int16)
    sidx_w = np.zeros((NCORES, P, T // 16), np.int16)
    sval_w = np.zeros((NCORES, P, T // 128), np.float32)
    offs = np.concatenate([[0], np.cumsum(caps.reshape(-1))]).astype(np.int64)

    for c in range(NCORES):
        r, coi, v, ch, rnd, parity = per_core[c]
        gi = np.zeros(T, np.int16)
        si = np.zeros(T, np.int16)
        sv = np.zeros(T, np.float32)
        s = 0
        for k in range(NCHUNK):
            for j in range(MAXROUNDS):
                n = int(sizes_all[c, k, j])
                d0 = int(offs[k * MAXROUNDS + j])
                gi[d0:d0 + n] = coi[s:s + n]
                si[d0:d0 + n] = r[s:s + n] * 2 + parity[s:s + n]
                sv[d0:d0 + n] = v[s:s + n]
                s += n
        # wrap for the SWDGE index layout: idx i -> [i%16, i//16], replicated x8
        gidx_w[c] = np.tile(gi.reshape(-1, 16).T, (8, 1))
        sidx_w[c] = np.tile(si.reshape(-1, 16).T, (8, 1))
        # vals layout: edge i -> [i%128, i//128]
        sval_w[c] = sv.reshape(-1, 128).T

    return tuple(calls), gidx_w, sidx_w, sval_w


def _build(calls):
    from concourse import bass, mybir, tile, bacc
    from concourse.masks import make_identity

    f32 = mybir.dt.float32
    i16 = mybir.dt.int16
    T = int(sum(n for _, _, n in calls))
    T = max(e0 + n for _, e0, n in calls)

    nc = bacc.Bacc("TRN2", target_bir_lowering=False, debug=False,
                   num_swdge_queues=4)

    x_d = nc.dram_tensor("x", [N_NODES, D], f32, kind="ExternalInput")
    xT_d = nc.dram_tensor("xlocT", [D, RPC], f32, kind="ExternalInput")
    w_d = nc.dram_tensor("W", [2 * D, OUT], f32, kind="ExternalInput")
    gidx_d = nc.dram_tensor("gidx", [P, T // 16], i16, kind="ExternalInput")
    sidx_d = nc.dram_tensor("sidx", [P, T // 16], i16, kind="ExternalInput")
    sval_d = nc.dram_tensor("svals", [P, T // 128], f32, kind="ExternalInput")
    out_d = nc.dram_tensor("out", [RPC, OUT], f32, kind="ExternalOutput")
    h_d = nc.dram_tensor("h_acc", [2 * H_PAD, D], f32)   # parity-striped

    relu = mybir.ActivationFunctionType.Relu

    with tile.TileContext(nc) as tc:
        with tc.tile_pool(name="const", bufs=1) as constp, \
             tc.tile_pool(name="mess", bufs=3) as messp, \
             tc.tile_pool(name="meta", bufs=3) as metap, \
             tc.tile_pool(name="dense", bufs=4) as densep, \
             tc.tile_pool(name="psum", bufs=2, space="PSUM") as psump:

            ident = constp.tile([P, P], f32)
            make_identity(nc, ident[:])
            w1 = constp.tile([D, OUT], f32)
            nc.sync.dma_start(out=w1[:], in_=w_d[:D, :])
            w2 = constp.tile([D, OUT], f32)
            nc.sync.dma_start(out=w2[:], in_=w_d[D:, :])

            # zero the striped h accumulator (2*H_PAD = 25600 rows)
            zblk = constp.tile([P, 8, D], f32)
            nc.vector.memset(zblk[:], 0.0)
            for b in range(2 * H_PAD // 1024):
                dst = h_d[b * 1024:(b + 1) * 1024, :].rearrange(
                    "(a p) d -> p a d", p=P)
                nc.scalar.dma_start(out=dst, in_=zblk[:])

            # ---- SpMM phase: gather -> scale -> scatter-add ----
            qrr = 0
            for k, e0, n in calls:
                ns = n // 128
                x_chunk = x_d[k * CHUNK:(k + 1) * CHUNK, :]
                gi = metap.tile([P, TILE_E // 16], i16, tag="gi")
                nc.sync.dma_start(
                    out=gi[:, :n // 16],
                    in_=gidx_d[:, e0 // 16:(e0 + n) // 16])
                mv = messp.tile([P, TILE_E // 128, D], f32, tag="mess")
                nc.gpsimd.dma_gather(
                    mv[:, :ns, :], x_chunk, gi[:, :n // 16], n, n, D,
                    queue_num=1 + (qrr % 3))
                qrr += 1
                sv = metap.tile([P, TILE_E // 128], f32, tag="sv")
                nc.sync.dma_start(
                    out=sv[:, :ns],
                    in_=sval_d[:, e0 // 128:(e0 + n) // 128])
                nc.vector.tensor_tensor(
                    out=mv[:, :ns, :], in0=mv[:, :ns, :],
                    in1=sv[:, :ns, None].to_broadcast([P, ns, D]),
                    op=mybir.AluOpType.mult)
                si = metap.tile([P, TILE_E // 16], i16, tag="si")
                nc.scalar.dma_start(
                    out=si[:, :n // 16],
                    in_=sidx_d[:, e0 // 16:(e0 + n) // 16])
                nc.gpsimd.dma_scatter_add(
                    h_d[:], mv[:, :ns, :], si[:, :n // 16], n, n, D)

            # ---- dense head: out = relu(h @ W1 + x @ W2) ----
            h_pairs = h_d[:].rearrange("(a two) d -> a two d", two=2)
            ngroups = (RPC + P - 1) // P       # 98 (97 full + one 84-row tail)
            for g in range(ngroups):
                g0 = g * P
                rsz = min(P, RPC - g0)
                hb = densep.tile([P, D], f32, tag="hb")
                nc.sync.dma_start(out=hb[:rsz, :],
                                  in_=h_pairs[g0:g0 + rsz, 0, :])
                hc = densep.tile([P, D], f32, tag="hc")
                nc.sync.dma_start(out=hc[:rsz, :],
                                  in_=h_pairs[g0:g0 + rsz, 1, :])
                nc.vector.tensor_add(out=hb[:rsz, :], in0=hb[:rsz, :],
                                     in1=hc[:rsz, :])
                pt = psump.tile([P, P], f32, tag="pt")
                nc.tensor.transpose(pt[:, :rsz], hb[:rsz, :], ident[:rsz, :rsz])
                hT = densep.tile([P, P], f32, tag="hT")
                nc.vector.tensor_copy(hT[:, :rsz], pt[:, :rsz])
                xT = densep.tile([P, P], f32, tag="xT")
                nc.sync.dma_start(out=xT[:, :rsz], in_=xT_d[:, g0:g0 + rsz])
                po = psump.tile([P, OUT], f32, tag="po")
                nc.tensor.matmul(po[:rsz, :], hT[:, :rsz], w1[:],
                                 start=True, stop=False)
                nc.tensor.matmul(po[:rsz, :], xT[:, :rsz], w2[:],
                                 start=False, stop=True)
                ob = densep.tile([P, OUT], f32, tag="ob")
                nc.scalar.activation(ob[:rsz, :], po[:rsz, :], relu)
                nc.scalar.dma_start(out=out_d[g0:g0 + rsz, :], in_=ob[:rsz, :])

    nc.compile()
    return nc


def _get_nc(calls):
    nc = _compiled.get(calls)
    if nc is None:
        nc = _build(calls)
        _compiled[calls] = nc
    return nc


def _make_in_maps(x, W, calls, gidx_w, sidx_w, sval_w):
    x = np.ascontiguousarray(np.asarray(x, np.float32))
    W = np.ascontiguousarray(np.asarray(W, np.float32))
    in_maps = []
    for c in range(NCORES):
        xloc = x[c * RPC:(c + 1) * RPC]
        in_maps.append({
            "x": x,
            "xlocT": np.ascontiguousarray(xloc.T),
            "W": W,
            "gidx": gidx_w[c],
            "sidx": sidx_w[c],
            "svals": sval_w[c],
        })
    return in_maps


def _install_trace_shims():
    """Make trace=True work in this container: provide antenv.axon_hooks
    (ctypes NTFF profiling via the axon PJRT .so) and stub the artifact
    upload (no bucket access here)."""
    import contextlib
    import ctypes
    import types

    try:
        import antenv.axon_hooks  # noqa: F401
        has_hooks = True
    except ImportError:
        has_hooks = False
    if not has_hooks:
        so_path = "/opt/axon/libaxon_pjrt.so"
        lib = ctypes.CDLL(so_path)
        if hasattr(lib, "axon_start_nrt_profile"):
            lib.axon_start_nrt_profile.argtypes = [
                ctypes.POINTER(ctypes.c_int64), ctypes.c_size_t]
            lib.axon_start_nrt_profile.restype = ctypes.c_int64
            lib.axon_stop_nrt_profile.argtypes = [ctypes.c_char_p]
            lib.axon_stop_nrt_profile.restype = ctypes.c_int64

            @contextlib.contextmanager
            def _hook(output_dir, device_ids):
                import jax
                jax.devices()
                if device_ids:
                    ids = (ctypes.c_int64 * len(device_ids))(*device_ids)
                    rc = lib.axon_start_nrt_profile(ids, len(device_ids))
                else:
                    rc = lib.axon_start_nrt_profile(None, 0)
                if rc != 0:
                    raise RuntimeError(f"axon_start_nrt_profile rc={rc}")
                try:
                    yield
                finally:
                    n = lib.axon_stop_nrt_profile(str(output_dir).encode())
                    if n <= 0:
                        print(f"ntff profile: rc={n} (no files?) at {output_dir}")

            mod = types.ModuleType("antenv.axon_hooks")
            mod.get_axon_ntff_profile_hook = lambda: _hook
            mod.set_axon_ntff_profile_hook = lambda h: None
            sys.modules["antenv.axon_hooks"] = mod

    import concourse.bass_utils as bu
    bu.upload_artifacts = lambda tmpdir: f"local:{tmpdir}"


def _run(x, adj_rows, adj_cols, adj_vals, W, trace=False):
    from concourse.bass_utils import run_bass_kernel_spmd
    if trace:
        try:
            _install_trace_shims()
        except Exception as e:  # tracing is best-effort
            print("trace shim install failed:", e)
    calls, gidx_w, sidx_w, sval_w = _prep(adj_rows, adj_cols, adj_vals)
    nc = _get_nc(calls)
    in_maps = _make_in_maps(x, W, calls, gidx_w, sidx_w, sval_w)
    res = run_bass_kernel_spmd(nc, in_maps, list(range(NCORES)), trace=trace)
    out = np.concatenate([res.results[c]["out"] for c in range(NCORES)], axis=0)
    return out, res


def kernel(x, adj_rows, adj_cols, adj_vals, W):
    out, _ = _run(x, adj_rows, adj_cols, adj_vals, W, trace=False)
    return out
